# revision 1
# baseline (speedup 1.0000x reference)
"""AttentionPairBias Trainium2 kernel (8 NeuronCores, SPMD over query rows).

Sharding: the 768 query rows are split 96-per-core. Each core computes the
full output rows for its query slice; the host concatenates.

Device-side math (per core), exact LN algebra with centered weights:
  Wz'' = w*Wz - colsum(w*Wz)/CZ   (folds the LN mean term into the weights)
  pair_bias = rstd * (zT @ Wz'') + mask_bias   (+ colsum(b*Wz) in the exp)
  attention in "T-domain": scoresT[k, q] per head, softmax over the k
  (partition) axis; denominator via a ones-column in the v matmul; the
  pair-bias is accumulated into the scores PSUM by an identity matmul.

The z contraction keeps the tiny weight matrix STATIONARY on the PE (the
moving operand is z, N=512) so the tensor engine is matmul-bound rather
than LDWEIGHTS-bound; results land as [32-row groups, ij] stacked four
deep in one PSUM bank and are transposed back to key-partition layout
with full-width PE transposes.

The host passes z pre-transposed to [CZ=128, kt, q, kin] (bf16, key-tile
major so attention can start before the whole z pass finishes), plus
zero-padded / folded weight layouts.
"""

import os
import sys
import numpy as np

sys.path.insert(0, "/opt/trn_rl_repo")
os.environ.setdefault("MYCRO_LOCAL_CACHE", "1")

from ml_dtypes import bfloat16

# ---- problem constants (hardcoded per the harness contract) ----
B, N, C, CZ, H, CH = 1, 768, 384, 128, 16, 24
NCORES = 8
NQ = N // NCORES          # 96 query rows per core
CHP = 32                  # padded per-head width
HP = H * CHP              # 512 padded hc
EPS = 1e-5
INF = 1e9
KT = N // 128             # 6 key tiles
QG = 16                   # query rows per z-chunk
NQG = NQ // QG            # 6 query groups
NCHUNK = KT * NQG         # 36 chunks, key-tile major
NBLK = 4                  # 512-wide moving blocks per chunk

_CACHE = {}


def _build_program():
    from contextlib import ExitStack
    import concourse.bass as bass
    import concourse.tile as tile
    from concourse import bacc, mybir

    f32 = mybir.dt.float32
    b16 = mybir.dt.bfloat16
    AF = mybir.ActivationFunctionType
    OP = mybir.AluOpType

    nc = bacc.Bacc("TRN2", target_bir_lowering=False, debug=False)

    # ---- DRAM I/O ----
    zt_d = nc.dram_tensor("zt", [CZ, KT * NQ * 128], b16, kind="ExternalInput")
    a_d = nc.dram_tensor("a_full", [N, C], f32, kind="ExternalInput")
    aq_d = nc.dram_tensor("a_q", [NQ, C], f32, kind="ExternalInput")
    wq_d = nc.dram_tensor("wq", [C, HP], b16, kind="ExternalInput")
    wk_d = nc.dram_tensor("wk", [C, HP], b16, kind="ExternalInput")
    wg_d = nc.dram_tensor("wg", [C, HP], b16, kind="ExternalInput")
    wv_d = nc.dram_tensor("wv", [C, C], b16, kind="ExternalInput")
    wo_d = nc.dram_tensor("wo", [HP, C], b16, kind="ExternalInput")
    bg_d = nc.dram_tensor("bg", [1, HP], b16, kind="ExternalInput")
    # host-folded z weights: col 0:16 = w*Wz - colsum(w*Wz)/CZ, col 16 = 1
    wza_d = nc.dram_tensor("wza", [CZ, 32], b16, kind="ExternalInput")
    # second stationary: col 17 = 1 (sum of squares), rest 0
    wzb_d = nc.dram_tensor("wzb", [CZ, 32], b16, kind="ExternalInput")
    tb_d = nc.dram_tensor("tbb", [128, H], f32, kind="ExternalInput")
    bqr_d = nc.dram_tensor("bqr", [1, HP], b16, kind="ExternalInput")
    bkr_d = nc.dram_tensor("bkr", [1, HP], b16, kind="ExternalInput")
    bvr_d = nc.dram_tensor("bvr", [1, C], b16, kind="ExternalInput")
    bo_d = nc.dram_tensor("bob", [128, C], f32, kind="ExternalInput")
    mask_d = nc.dram_tensor("maskt", [128, KT], f32, kind="ExternalInput")
    id_d = nc.dram_tensor("ident", [128, 128], b16, kind="ExternalInput")
    out_d = nc.dram_tensor("out", [NQ, C], f32, kind="ExternalOutput")

    with tile.TileContext(nc) as tc, ExitStack() as ctx:
        const = ctx.enter_context(tc.tile_pool(name="const", bufs=1))

        # ------------- constant loads (scalar-engine HWDGE ring; z and the
        # output go on the sync ring) ------
        wzaug = const.tile([CZ, 32], b16)
        nc.scalar.dma_start(wzaug, wza_d[:, :])
        wzsq = const.tile([CZ, 32], b16)
        nc.scalar.dma_start(wzsq, wzb_d[:, :])
        sb_mask = const.tile([128, KT], f32)
        nc.scalar.dma_start(sb_mask, mask_d[:, :])
        sb_id = const.tile([128, 128], b16)
        nc.scalar.dma_start(sb_id, id_d[:, :])
        tb_b = const.tile([128, H], f32)
        nc.scalar.dma_start(tb_b, tb_d[:, :])
        bo_b = const.tile([128, C], f32)
        nc.scalar.dma_start(bo_b, bo_d[:, :])
        sb_bq = const.tile([1, HP], b16)
        nc.scalar.dma_start(sb_bq, bqr_d[:, :])
        sb_bk = const.tile([1, HP], b16)
        nc.scalar.dma_start(sb_bk, bkr_d[:, :])
        sb_bv = const.tile([1, C], b16)
        nc.scalar.dma_start(sb_bv, bvr_d[:, :])
        sb_bg = const.tile([1, HP], b16)
        nc.scalar.dma_start(sb_bg, bg_d[:, :])

        a_sb = []
        for it in range(7):
            t = const.tile([128, C], f32, name=f"a{it}")
            if it < 6:
                nc.scalar.dma_start(t, a_d[128 * it:128 * (it + 1), :])
            else:
                nc.scalar.dma_start(t[0:NQ, :], aq_d[:, :])
            a_sb.append(t)

        wq_sb = []
        wk_sb = []
        wg_sb = []
        wv_sb = []
        for c in range(3):
            t = const.tile([128, HP], b16, name=f"wq{c}")
            nc.scalar.dma_start(t, wq_d[128 * c:128 * (c + 1), :])
            wq_sb.append(t)
            t = const.tile([128, HP], b16, name=f"wk{c}")
            nc.scalar.dma_start(t, wk_d[128 * c:128 * (c + 1), :])
            wk_sb.append(t)
            t = const.tile([128, HP], b16, name=f"wg{c}")
            nc.scalar.dma_start(t, wg_d[128 * c:128 * (c + 1), :])
            wg_sb.append(t)
            t = const.tile([128, C], b16, name=f"wv{c}")
            nc.scalar.dma_start(t, wv_d[128 * c:128 * (c + 1), :])
            wv_sb.append(t)
        wo_sb = []
        for c in range(4):
            t = const.tile([128, C], b16, name=f"wo{c}")
            nc.scalar.dma_start(t, wo_d[128 * c:128 * (c + 1), :])
            wo_sb.append(t)

        # small derived constants
        ones_row_b96 = const.tile([1, NQ], b16)
        nc.vector.memset(ones_row_b96, 1.0)
        ones_row_b768 = const.tile([1, N], b16)
        nc.vector.memset(ones_row_b768, 1.0)
        ones_f32c = const.tile([128, CHP], f32)
        nc.vector.memset(ones_f32c, 1.0)
        eps_t = const.tile([128, 1], f32)
        nc.vector.memset(eps_t, EPS)
        # mask bias per key partition (folded into the stored pair-bias)
        mb = const.tile([128, KT], f32)
        nc.vector.tensor_scalar(mb, sb_mask, 1.0, INF, OP.subtract, OP.mult)

        # phase-B SBUF pools + PSUM pools open before phase A's (LIFO order:
        # A's pools release first, then B's, then phase C allocates)
        zpool = ctx.enter_context(tc.tile_pool(name="zpool", bufs=3))
        sqpool = ctx.enter_context(tc.tile_pool(name="sqpool", bufs=2))
        sbpool = ctx.enter_context(tc.tile_pool(name="sbp", bufs=2))
        zsm = ctx.enter_context(tc.tile_pool(name="zsmall", bufs=2))
        b_stack = ExitStack()
        psAp = b_stack.enter_context(tc.tile_pool(name="psA", bufs=3, space="PSUM"))
        psTp = b_stack.enter_context(tc.tile_pool(name="psT", bufs=2, space="PSUM"))

        # ------------- phase A: LN(a) + projections -------------
        a_stack = ExitStack()
        an_t = []
        apool = a_stack.enter_context(tc.tile_pool(name="apool", bufs=2))
        for it in range(7):
            p = 128 if it < 6 else NQ
            at = a_sb[it]
            stats = apool.tile([128, 6], f32, tag="stats")
            nc.vector.bn_stats(stats[0:p, :], at[0:p, :])
            mv = apool.tile([128, 2], f32, tag="mv")
            nc.vector.bn_aggr(mv[0:p, :], stats[0:p, :])
            stdv = apool.tile([128, 1], f32, tag="stdv")
            nc.scalar.activation(
                stdv[0:p, :], mv[0:p, 1:2], AF.Sqrt, bias=eps_t[0:p, :]
            )
            rstd = apool.tile([128, 1], f32, tag="rstd")
            nc.vector.reciprocal(rstd[0:p, :], stdv[0:p, :])
            ant = const.tile([128, C], b16, name=f"an{it}")
            nc.vector.tensor_scalar(
                ant[0:p, :], at[0:p, :], mv[0:p, 0:1], rstd[0:p, :],
                OP.subtract, OP.mult,
            )
            an_t.append(ant)

        anT = [const.tile([128, N], b16, name=f"anT{c}") for c in range(3)]
        anTq = [const.tile([128, NQ], b16, name=f"anTq{c}") for c in range(3)]
        pstr = a_stack.enter_context(tc.tile_pool(name="pstr", bufs=1, space="PSUM"))
        for it in range(6):
            for c in range(3):
                tp = pstr.tile([128, 128], b16, tag="tp")
                nc.tensor.transpose(tp, an_t[it][:, 128 * c:128 * (c + 1)], sb_id)
                nc.vector.tensor_copy(anT[c][:, 128 * it:128 * (it + 1)], tp)
        for c in range(3):
            tp = pstr.tile([128, NQ], b16, tag="tp", name="tpq")
            nc.tensor.transpose(
                tp, an_t[6][0:NQ, 128 * c:128 * (c + 1)], sb_id[0:NQ, 0:NQ]
            )
            nc.vector.tensor_copy(anTq[c], tp)

        kTt = [const.tile([128, N], b16, name=f"kT{j}") for j in range(4)]
        v_aug = [const.tile([128, H, CHP], b16, name=f"vaug{t}") for t in range(KT)]
        qTt = [const.tile([128, NQ], b16, name=f"qT{j}") for j in range(4)]
        gTt = [const.tile([128, NQ], b16, name=f"gT{j}") for j in range(4)]
        psp = a_stack.enter_context(tc.tile_pool(name="psproj", bufs=1, space="PSUM"))
        for j in range(4):
            for half in range(2):
                hw = 384
                kps = psp.tile([128, 384], f32, tag="kps", bufs=1, name=f"kps{j}_{half}")
                for c in range(3):
                    nc.tensor.matmul(
                        kps,
                        wk_sb[c][:, 128 * j:128 * (j + 1)],
                        anT[c][:, hw * half:hw * (half + 1)],
                        start=(c == 0), stop=False,
                    )
                nc.tensor.matmul(
                    kps, sb_bk[0:1, 128 * j:128 * (j + 1)],
                    ones_row_b768[0:1, hw * half:hw * (half + 1)],
                    start=False, stop=True,
                )
                nc.vector.tensor_copy(kTt[j][:, hw * half:hw * (half + 1)], kps)
        for t in range(KT):
            vps = psp.tile([128, C], f32, tag="pps", name="vps", bufs=1)
            for c in range(3):
                nc.tensor.matmul(
                    vps, anT[c][:, 128 * t:128 * (t + 1)], wv_sb[c],
                    start=(c == 0), stop=False,
                )
            nc.tensor.matmul(
                vps, ones_row_b768[0:1, 0:128], sb_bv,
                start=False, stop=True,
            )
            nc.gpsimd.memset(v_aug[t], 0.0)
            nc.gpsimd.memset(v_aug[t][:, :, 0:1], 1.0)
            nc.vector.tensor_copy(
                v_aug[t][:, :, 1:CH + 1],
                vps.rearrange("p (h c) -> p h c", h=H),
            )
        for j in range(4):
            qps = psp.tile([128, NQ], f32, tag="pps", name="qps", bufs=1)
            for c in range(3):
                nc.tensor.matmul(
                    qps, wq_sb[c][:, 128 * j:128 * (j + 1)], anTq[c],
                    start=(c == 0), stop=False,
                )
            nc.tensor.matmul(
                qps, sb_bq[0:1, 128 * j:128 * (j + 1)], ones_row_b96,
                start=False, stop=True,
            )
            nc.scalar.activation(qTt[j], qps, AF.Copy, scale=float(CH) ** -0.5)
            gps = psp.tile([128, NQ], f32, tag="pps", name="gps", bufs=1)
            for c in range(3):
                nc.tensor.matmul(
                    gps, wg_sb[c][:, 128 * j:128 * (j + 1)], anTq[c],
                    start=(c == 0), stop=False,
                )
            nc.tensor.matmul(
                gps, sb_bg[0:1, 128 * j:128 * (j + 1)], ones_row_b96,
                start=False, stop=True,
            )
            nc.scalar.activation(gTt[j], gps, AF.Sigmoid)

        # ------------- phase B: pair-bias from z (key-tile major) ---------
        # bias_sb layout: [k=128, kt, q, h]
        bias_sb = const.tile([128, KT, NQ, H], b16)
        FW = QG * 128  # 2048 free elems per chunk
        for chk in range(NCHUNK):
            kt, qg = chk // NQG, chk % NQG
            zt_t = zpool.tile([128, FW], b16, tag="zt")
            nc.sync.dma_start(zt_t, zt_d[:, FW * chk:FW * (chk + 1)])
            sq_t = sqpool.tile([128, FW], b16, tag="sq")
            if chk % 2 == 0:
                nc.scalar.square(sq_t, zt_t)
            else:
                nc.vector.tensor_tensor(sq_t, zt_t, zt_t, OP.mult)
            # contraction: weights stationary, z moving; results stacked
            # 4 blocks deep (32-row groups) in one PSUM bank
            psA = psAp.tile([128, 512], f32, tag="psA")
            for b in range(NBLK):
                nc.tensor.matmul(
                    psA[32 * b:32 * b + 32, :], wzaug,
                    zt_t[:, 512 * b:512 * (b + 1)],
                    start=True, stop=False,
                    tile_position=(0, 32 * b), skip_group_check=True,
                )
            for b in range(NBLK):
                nc.tensor.matmul(
                    psA[32 * b:32 * b + 32, :], wzsq,
                    sq_t[:, 512 * b:512 * (b + 1)],
                    start=False, stop=True,
                    tile_position=(0, 32 * b), skip_group_check=True,
                )
            sbA = sbpool.tile([128, 512], b16, tag="sbA")
            if chk % 2 == 0:
                nc.vector.tensor_copy(sbA, psA)
            else:
                nc.scalar.copy(sbA, psA)
            # transpose back to key-partition layout: psT[kin, (s, b, r)]
            psT = psTp.tile([128, NBLK, NBLK, 32], b16, tag="psT")
            for s in range(NBLK):
                nc.tensor.transpose(
                    psT[:, s, :, :].rearrange("p a b -> p (a b)"),
                    sbA[:, 128 * s:128 * (s + 1)], sb_id,
                )
            # stats + bias on full-width batched views
            S = psT[:, :, :, H]                 # [128, s, b]
            Q = psT[:, :, :, H + 1]
            mu = zsm.tile([128, NBLK, NBLK], f32, tag="mu")
            nc.vector.tensor_scalar(mu, S, 1.0 / CZ, None, OP.mult)
            v1 = zsm.tile([128, NBLK, NBLK], f32, tag="v1")
            nc.vector.tensor_tensor(v1, mu, mu, OP.mult)
            var = zsm.tile([128, NBLK, NBLK], f32, tag="var")
            nc.vector.scalar_tensor_tensor(
                var, Q, 1.0 / CZ, v1, OP.mult, OP.subtract
            )
            stdv = zsm.tile([128, NBLK, NBLK], f32, tag="stdv")
            nc.scalar.activation(stdv, var, AF.Sqrt, bias=eps_t)
            rstd = zsm.tile([128, NBLK, NBLK], f32, tag="rstd")
            nc.vector.reciprocal(rstd, stdv)
            tbig = zsm.tile([128, NBLK, NBLK, H], f32, tag="tbig")
            nc.vector.tensor_tensor(
                tbig, psT[:, :, :, 0:H],
                rstd[:, :, :, None].broadcast_to([128, NBLK, NBLK, H]),
                OP.mult,
            )
            # bias_sb[:, kt, qg*QG + 4*b + s, :] = tbig[:, s, b, :] + mb[kt]
            outap = bias_sb[:, kt, qg * QG:(qg + 1) * QG, :].rearrange(
                "p (b s) h -> p s b h", s=NBLK
            )
            nc.vector.tensor_scalar(
                outap, tbig, mb[:, kt:kt + 1], None, OP.add,
            )

        # ------------- phase C: attention -------------
        a_stack.close()
        b_stack.close()
        goT = [const.tile([128, NQ], b16, name=f"goT{c}") for c in range(4)]
        for c in range(4):
            nc.gpsimd.memset(goT[c], 0.0)
        KG = 3   # key tiles per scores group
        with (
            tc.tile_pool(name="scps", bufs=3, space="PSUM") as scps,
            tc.tile_pool(name="otps", bufs=3, space="PSUM") as otps,
            tc.tile_pool(name="rbps", bufs=1, space="PSUM") as rbps,
            tc.tile_pool(name="pexp", bufs=4) as pexp,
            tc.tile_pool(name="rcpool", bufs=2) as rcpool,
            tc.tile_pool(name="tmppool", bufs=2) as tmppool,
        ):
            for h in range(H):
                cn, j = h // 4, h % 4
                jb = 32 * j
                oT = otps.tile([128, NQ], f32, tag="oT")
                for kg in range(KT // KG):
                    sc = scps.tile([128, KG, NQ], f32, tag="sc")
                    # pair-bias (+mask) via identity matmul FIRST (start=True
                    # claims the whole psum zero-region once), then the score
                    # matmuls accumulate on top.
                    nc.tensor.matmul(
                        sc.rearrange("p a b -> p (a b)"),
                        sb_id,
                        bias_sb[:, KG * kg:KG * (kg + 1), :, h],
                        start=True, stop=False,
                        tile_position=(0, 0), skip_group_check=True,
                    )
                    for ks in range(KG):
                        kt = KG * kg + ks
                        nc.tensor.matmul(
                            sc[:, ks, :],
                            kTt[cn][jb:jb + CHP, 128 * kt:128 * (kt + 1)],
                            qTt[cn][jb:jb + CHP, :],
                            start=False, stop=(ks == KG - 1),
                            tile_position=(jb, 0), skip_group_check=True,
                        )
                    p_t = pexp.tile([128, KG, NQ], b16, tag="pt")
                    nc.scalar.activation(
                        p_t, sc, AF.Exp, bias=tb_b[:, h:h + 1]
                    )
                    for ks in range(KG):
                        kt = KG * kg + ks
                        nc.tensor.matmul(
                            oT[jb:jb + CHP, :], v_aug[kt][:, h, :], p_t[:, ks, :],
                            start=(kt == 0), stop=(kt == KT - 1),
                            tile_position=(0, jb), skip_group_check=True,
                        )
                recip_t = rcpool.tile([128, NQ], f32, tag="recip")
                nc.vector.reciprocal(recip_t[jb:jb + 1, :], oT[jb:jb + 1, :])
                rb = rbps.tile([128, NQ], f32, tag="rb")
                nc.tensor.matmul(
                    rb[jb:jb + CHP, :], ones_f32c[jb:jb + 1, :],
                    recip_t[jb:jb + 1, :],
                    tile_position=(jb, jb), skip_group_check=True,
                )
                tmp = tmppool.tile([128, NQ], f32, tag="tmp")
                nc.vector.tensor_tensor(
                    tmp[jb:jb + CHP, :], oT[jb:jb + CHP, :],
                    gTt[cn][jb:jb + CHP, :], OP.mult,
                )
                nc.vector.tensor_tensor(
                    goT[cn][jb:jb + CHP, :], tmp[jb:jb + CHP, :],
                    rb[jb:jb + CHP, :], OP.mult,
                )

            with tc.tile_pool(name="psfin", bufs=1, space="PSUM") as psf:
                ops = psf.tile([NQ, C], f32)
                for cn in range(4):
                    nc.tensor.matmul(
                        ops, goT[cn], wo_sb[cn], start=(cn == 0),
                        stop=(cn == 3), skip_group_check=True,
                    )
                out_sb = const.tile([NQ, C], f32)
                nc.vector.tensor_tensor(out_sb, ops, bo_b[0:NQ, :], OP.add)
                nc.sync.dma_start(out_d[:, :], out_sb)

    nc.compile()
    return nc


def _get_program():
    if "nc" not in _CACHE:
        _CACHE["nc"] = _build_program()
    return _CACHE["nc"]


def _pad_heads_cols(w, off):
    out = np.zeros((C, H, CHP), np.float32)
    out[:, :, off:off + CH] = np.asarray(w, np.float32).reshape(C, H, CH)
    return out.reshape(C, HP).astype(bfloat16)


def _host_inputs(inputs):
    a = np.asarray(inputs["a"], np.float32)
    z = np.asarray(inputs["z"], np.float32)
    mask = np.asarray(inputs["mask"], np.float32)
    Wz = np.asarray(inputs["Wz"], np.float32)
    Wo = np.asarray(inputs["Wo"], np.float32)
    bg = np.asarray(inputs["bg"], np.float32)
    lnzw = np.asarray(inputs["ln_z_w"], np.float32)
    lnzb = np.asarray(inputs["ln_z_b"], np.float32)
    lnaw = np.asarray(inputs["ln_a_w"], np.float32)
    lnab = np.asarray(inputs["ln_a_b"], np.float32)
    # fold LN(a)'s elementwise w into the projection weights; its b becomes
    # per-projection bias rows added via K=1 matmuls on-device
    Wq = lnaw[:, None] * np.asarray(inputs["Wq"], np.float32)
    Wk = lnaw[:, None] * np.asarray(inputs["Wk"], np.float32)
    Wg = lnaw[:, None] * np.asarray(inputs["Wg"], np.float32)
    Wv = lnaw[:, None] * np.asarray(inputs["Wv"], np.float32)
    bq = lnab @ np.asarray(inputs["Wq"], np.float32)
    bk = lnab @ np.asarray(inputs["Wk"], np.float32)
    bv = lnab @ np.asarray(inputs["Wv"], np.float32)
    bgf = bg + lnab @ np.asarray(inputs["Wg"], np.float32)

    wo_p = np.zeros((H, CHP, C), np.float32)
    wo_p[:, 1:CH + 1, :] = Wo.reshape(H, CH, C)
    bg_p = np.zeros((H, CHP), np.float32)
    bg_p[:, 1:CH + 1] = bgf.reshape(H, CH)
    def _pad_row(v, off):
        out = np.zeros((H, CHP), np.float32)
        out[:, off:off + CH] = v.reshape(H, CH)
        return out.reshape(1, HP).astype(bfloat16)

    # folded z-weight stationaries (bf16, consistency: center the bf16 values)
    wzp = (lnzw[:, None] * Wz).astype(bfloat16).astype(np.float32)
    wza = np.zeros((CZ, 32), np.float32)
    wza[:, 0:H] = wzp - wzp.sum(axis=0, keepdims=True) / CZ
    wza[:, H] = 1.0
    wzb = np.zeros((CZ, 32), np.float32)
    wzb[:, H + 1] = 1.0
    tb = (lnzb[:, None] * Wz).sum(axis=0)          # [H]

    shared = {
        "a_full": np.ascontiguousarray(a[0]),
        "wq": _pad_heads_cols(Wq, 0),
        "wk": _pad_heads_cols(Wk, 0),
        "wg": _pad_heads_cols(Wg, 1),
        "wv": Wv.astype(bfloat16),
        "wo": wo_p.reshape(HP, C).astype(bfloat16),
        "bg": bg_p.reshape(1, HP).astype(bfloat16),
        "bqr": _pad_row(bq, 0),
        "bkr": _pad_row(bk, 0),
        "bvr": bv.reshape(1, C).astype(bfloat16),
        "wza": wza.astype(bfloat16),
        "wzb": wzb.astype(bfloat16),
        "tbb": np.ascontiguousarray(np.broadcast_to(tb, (128, H))),
        "bob": np.ascontiguousarray(
            np.broadcast_to(np.asarray(inputs["bo"], np.float32), (128, C))),
        "maskt": np.ascontiguousarray(mask[0].reshape(KT, 128).T),
        "ident": np.eye(128, dtype=bfloat16),
    }
    in_maps = []
    zb = z[0].astype(bfloat16)  # [N(q), N(k), CZ] bf16
    for core in range(NCORES):
        qs = slice(NQ * core, NQ * (core + 1))
        # [CZ, kt, q, kin] key-tile-major transposed layout
        zt = zb[qs].transpose(2, 1, 0).reshape(CZ, KT, 128, NQ)
        zt = np.ascontiguousarray(zt.transpose(0, 1, 3, 2)).reshape(CZ, -1)
        m = dict(shared)
        m["zt"] = zt
        m["a_q"] = np.ascontiguousarray(a[0, qs])
        in_maps.append(m)
    return in_maps


def _run(inputs, trace=False):
    from concourse.bass_utils import run_bass_kernel_spmd

    nc = _get_program()
    in_maps = _host_inputs(inputs)
    res = run_bass_kernel_spmd(
        nc, in_maps, core_ids=list(range(NCORES)), trace=trace
    )
    rows = [res.results[i]["out"] for i in range(NCORES)]
    out = np.concatenate(rows, axis=0).reshape(B, N, C).astype(np.float32)
    return out, res


def kernel(**inputs):
    out, _ = _run(inputs, trace=False)
    return out

